# revision 26
# baseline (speedup 1.0000x reference)
"""Trainium2 Bass kernel for a 16-head self-attention block.

Model (matches the nn.Module reference):
    q = x @ Wq + bq; k = x @ Wk + bk; v = x @ Wv + bv   (per-head split, Hd=64)
    attn = softmax(q k^T / sqrt(Hd)); out = (attn v) @ Wo + bo
Shapes: x [2, 2048, 1024], 16 heads, head dim 64.

Sharding (8 cores): core = (batch b in {0,1}) x (head-group g in {0..3});
each core owns 4 heads of one batch element. Inputs are sliced on the host;
each core returns a partial y^T = (attended_g @ Wo_g)^T which the host sums
over the 4 head-groups per batch.

Per-core design (float32r matmuls = fp32 bytes on the fast PE path):
  - Host passes xT = x[b]^T so projections need no on-device transpose.
  - Scores are computed transposed, S^T[key, q] = K_h Q_h^T, so softmax's
    exp runs straight out of PSUM on the Scalar engine and A = P V consumes
    P^T with no transpose anywhere. Two heads of a pair share each score
    matmul slab via PE row groups (K=64 at row offsets 0/64 — legal with
    f32r; column tile offsets are not).
  - softmax skips the max subtraction (mathematically identical; scores are
    O(5) here and ACT exp is <=2 ULP on [-10,10]).
  - The Scalar engine runs ONLY the exp stream (the critical path at ~1.3us
    per [128,1024] eviction); K/Q projection evictions (+bias) run on DVE
    via tensor_scalar.
  - P row sums come from ones columns embedded in the V slab, placed so each
    head's sums row lands on a 32-aligned PSUM partition: the even head uses
    lhsT [d(64)|1] (M=65, sums at row 64); the odd head uses a 128-wide
    window [.|1@32|.|d@64..127] (M=128, sums at row 32, dims at rows 64-127,
    interleaved junk columns produce unread PSUM rows). This lets BOTH heads
    normalize with aligned K=1 ones-broadcast matmuls and lane-aligned DVE
    multiplies directly out of PSUM — no SBUF staging copies, no sums-row
    DMA, no odd-head partition-shift DMA.
  - 1/sums via vector.reciprocal_approx_fast (~51 ULP, ~5x faster than
    reciprocal; softmax denominators are benign).
  - 1/sqrt(Hd) is folded into Wq (and bq) on the host; bv and bo are folded
    in exactly on the host: y += bo + bv @ Wo (softmax rows sum to 1).
"""

import numpy as np

import concourse.bass as bass
import concourse.tile as tile
from concourse import bacc
from concourse import mybir

P = 128          # partitions
S = 2048         # sequence length
D = 1024         # model dim
H = 16           # total heads
HD = 64          # head dim
G = 4            # heads per core
GD = G * HD      # 256 head-group dims per core
NQB = 4          # query blocks
QB = S // NQB    # 512
NKC = S // P     # 16 key chunks
NDC = D // P     # 8 contraction chunks
VW = 192         # per-pair V slab width: [d0(64),1,junk(31),1,junk(31),d1(64)]
F32 = mybir.dt.float32
F32R = mybir.dt.float32r

TRACE = False
LAST_RESULTS = None


def _build_nc(nqb=NQB, do_attn=True, do_exp=True, do_outproj=True,
              do_norm=True, do_proj=True, do_dma=True):
    nc = bacc.Bacc(trn_type="TRN2")
    xT = nc.dram_tensor("xT", [D, S], F32R, kind="ExternalInput")
    wq = nc.dram_tensor("wq", [D, GD], F32R, kind="ExternalInput")
    wk = nc.dram_tensor("wk", [D, GD], F32R, kind="ExternalInput")
    wv = nc.dram_tensor("wv", [D, GD], F32R, kind="ExternalInput")
    wo = nc.dram_tensor("wo", [GD, D], F32R, kind="ExternalInput")
    bias = nc.dram_tensor("bias", [P, 4], F32, kind="ExternalInput")
    cst = nc.dram_tensor("cst", [P, 768], F32R, kind="ExternalInput")
    yT = nc.dram_tensor("yT", [D, S], F32, kind="ExternalOutput")

    Exp = mybir.ActivationFunctionType.Exp

    with tile.TileContext(nc) as tc, \
         tc.tile_pool(name="sb", bufs=1) as sb, \
         tc.tile_pool(name="pt", bufs=3) as ptp, \
         tc.tile_pool(name="attnp", bufs=5) as atp, \
         tc.tile_pool(name="ysbp", bufs=2) as ysbp, \
         tc.tile_pool(name="bcp", bufs=2) as bcp, \
         tc.tile_pool(name="tiny", bufs=3) as tnp, \
         tc.tile_pool(name="ps_s", bufs=2, space="PSUM") as ps_s, \
         tc.tile_pool(name="ps_av", bufs=2, space="PSUM") as ps_av, \
         tc.tile_pool(name="ps_y", bufs=2, space="PSUM") as ps_y:

        # ---- persistent SBUF tensors
        wq_sb = sb.tile([P, NDC, GD], F32R, tag="wq")
        wk_sb = sb.tile([P, NDC, GD], F32R, tag="wk")
        wv_sb = sb.tile([P, NDC, GD], F32R, tag="wv")
        wo_sb = sb.tile([P, 2, D], F32R, tag="wo")   # [pair-dims, pair, out-dim]
        bias_sb = sb.tile([P, 4], F32, tag="bias")
        scratch = sb.tile([P, 1], F32, tag="scratch")
        cst_sb = sb.tile([P, 768], F32R, tag="cst")
        ones_col = cst_sb[:, 0:1]                    # [128, 1] ones
        ones_even = cst_sb[64:65, 640:640 + P]       # [1, 128] ones at row 64
        ones_odd = cst_sb[0:1, 640:640 + P]          # [1, 128] ones at row 0
        ones4 = cst_sb[:, 740:744]                   # [128, 4] ones
        x_sb = [sb.tile([P, S], F32R, tag=f"x{d}", name=f"x{d}") for d in range(NDC)]
        kT = [sb.tile([P, S], F32R, tag=f"k{p}", name=f"k{p}") for p in range(2)]
        qT = [sb.tile([P, S], F32R, tag=f"q{p}", name=f"q{p}") for p in range(2)]
        # V slab per key chunk: [keys, pair, VW] with
        # cols 0:64 even dims, 64 ones, 96 ones, 128:192 odd dims.
        v_sb = [sb.tile([P, 2, VW], F32R, tag=f"v{c}", name=f"v{c}")
                for c in range(NKC)]

        # ---- input DMAs
        if do_dma:
            nc.sync.dma_start(out=wk_sb, in_=wk.rearrange("(o p) m -> p o m", p=P))
            nc.sync.dma_start(out=bias_sb, in_=bias[:, :])
            for d in range(3):
                nc.sync.dma_start(out=x_sb[d], in_=xT[d * P:(d + 1) * P, :])
            nc.sync.dma_start(out=wq_sb, in_=wq.rearrange("(o p) m -> p o m", p=P))
            for d in range(3, NDC):
                nc.sync.dma_start(out=x_sb[d], in_=xT[d * P:(d + 1) * P, :])
            nc.sync.dma_start(out=wv_sb, in_=wv.rearrange("(o p) m -> p o m", p=P))
            nc.sync.dma_start(out=cst_sb, in_=cst[:, :])
            nc.sync.dma_start(out=wo_sb, in_=wo.rearrange("(o p) m -> p o m", p=P))
        # warm the exp table set early so the ~2.7us load overlaps the prologue
        nc.scalar.activation(out=scratch, in_=ones_col.bitcast(F32), func=Exp)

        # pre-fill V slabs with 1.0 (DVE is idle during the DMA-bound
        # prologue): provides the per-head ones columns and initializes the
        # filler columns of each 128-wide odd-head window
        for c in range(NKC):
            nc.vector.memset(v_sb[c][:].bitcast(F32), 1.0)
        # persistent normalization staging; memset once so the batched
        # reciprocal's filler rows (33..63) read initialized memory
        sums_sb = sb.tile([P, QB], F32, tag="sums")
        rc_sb = sb.tile([P, QB], F32, tag="rcp")
        rcr_sb = sb.tile([P, QB], F32R, tag="rcr")
        nc.vector.memset(sums_sb[:], 1.0)

        # Pre-observe each weight DMA on the PE with a 1x1 dummy matmul, so
        # real matmuls never need two DMA-queue waits at once (walrus can't
        # encode >1 sync wait on an LDWEIGHTS).
        wtouch_ps = ps_y.tile([1, 4], F32, tag="y", name="wtouch")
        for i, w in enumerate((wk_sb, wv_sb, wq_sb)):
            nc.tensor.matmul(wtouch_ps[:, i:i + 1],
                             lhsT=w[0:1, 0, 0:1].bitcast(F32),
                             rhs=w[0:1, 0, 0:1].bitcast(F32),
                             start=True, stop=True)
        nc.tensor.matmul(wtouch_ps[:, 3:4],
                         lhsT=wo_sb[0:1, 0, 0:1].bitcast(F32),
                         rhs=wo_sb[0:1, 0, 0:1].bitcast(F32),
                         start=True, stop=True)

        # ---- projection emitters
        def emit_qk_group(w_sb, dst, bcol0, p, nb2):
            # one [128, 1024] output slab of K^T or Q^T; dst[p] [128, 2048]
            # rows 64*h2 hold head (2p+h2)'s 64 dims, columns are sequence.
            ps = ps_s.tile([P, 2, QB], F32, tag="s", name="qk_ps")
            for d in range(NDC):
                for half in range(2):
                    n0 = (2 * nb2 + half) * QB
                    nc.tensor.matmul(
                        ps[:, half],
                        lhsT=w_sb[:, d, p * P:(p + 1) * P],
                        rhs=x_sb[d][:, n0:n0 + QB],
                        start=(d == 0), stop=(d == NDC - 1))
            # evict with per-partition bias add on DVE (keeps ACT free for exp)
            nc.vector.tensor_scalar_add(
                out=dst[p][:, nb2 * 1024:(nb2 + 1) * 1024]
                    .rearrange("p (a b) -> p a b", a=2),
                in0=ps[:],
                scalar1=bias_sb[:, bcol0 + p:bcol0 + p + 1])

        def emit_v_chunk(c):
            ps = ps_y.tile([P, GD], F32, tag="y", name="v_ps")
            for d in range(NDC):
                nc.tensor.matmul(
                    ps[:],
                    lhsT=x_sb[d][:, c * P:(c + 1) * P],
                    rhs=wv_sb[:, d, :],
                    start=(d == 0), stop=(d == NDC - 1))
            pshd = ps[:].rearrange("p (a b d) -> p a b d", a=2, b=2)
            # even heads (h2=0) -> cols 0:64; odd heads (h2=1) -> cols 128:192
            # (the whole slab was memset to 1.0 in the prologue, providing the
            # ones columns at 64/96 and benign filler elsewhere)
            nc.vector.tensor_copy(out=v_sb[c][:, :, 0:HD], in_=pshd[:, :, 0, :])
            nc.vector.tensor_copy(out=v_sb[c][:, :, 2 * HD:3 * HD],
                                  in_=pshd[:, :, 1, :])

        if do_proj:
            # K first (its matmuls start as x chunks stream in), then the Q
            # halves needed by the first two query blocks, then V — so
            # attention can start while V chunks are still being projected.
            for p in range(2):
                for nb2 in range(2):
                    emit_qk_group(wk_sb, kT, 2, p, nb2)
            for p in range(2):
                emit_qk_group(wq_sb, qT, 0, p, 0)

        # ---- attention + output projection: per query block, head pairs
        # processed sequentially (pass p covers heads 2p, 2p+1). The output
        # projection of block qb is emitted a few chunks into block qb+1 so
        # its matmuls fill PE slack instead of stalling the exp stream.
        pending_outproj = []
        for qb in range(nqb if do_attn else 0):
            q0 = qb * QB
            attn = []
            for p in range(2):
                # av0: even head, M=65, dims rows 0-63, sums row 64.
                # av1: odd head, M=128, sums row 0, dims rows 64-127
                # (window col 0 is the even head's ones column).
                av0 = ps_av.tile([P, QB], F32, tag="av", name="av0")
                av1 = ps_av.tile([P, QB], F32, tag="av", name="av1")

                def emit_av(pt, c, av0=av0, av1=av1, p=p):
                    nc.tensor.matmul(
                        av0[0:HD + 1, :],
                        lhsT=v_sb[c][:, p, 0:HD + 1],
                        rhs=pt[:, 0],
                        start=(c == 0), stop=(c == NKC - 1))
                    nc.tensor.matmul(
                        av1[:, :],
                        lhsT=v_sb[c][:, p, HD:HD + P],
                        rhs=pt[:, 1],
                        start=(c == 0), stop=(c == NKC - 1))

                # software-pipelined by one chunk: scores(c)+exp(c) are
                # emitted BEFORE av(c-1), so in the PE's in-order queue the
                # (independent) score matmuls of chunk c run while the
                # Scalar engine still computes exp(c-1); the exp stream then
                # never waits on the PE and runs back-to-back.
                pending_av = None
                for c in range(NKC):
                    if do_proj and qb == 0 and p == 0:
                        emit_v_chunk(c)   # V streams in just ahead of its AV
                    if pending_outproj and p == 0 and c >= 2:
                        # one output-projection m-chunk per key chunk: its 2
                        # matmuls (~0.9us) fit the per-chunk PE slack, where a
                        # single clump would stall the exp stream for ~8us
                        pending_outproj.pop(0)()
                    c0 = c * P
                    s_ps = ps_s.tile([P, 2, QB], F32, tag="s")
                    for h2 in range(2):
                        base = HD * h2
                        nc.tensor.matmul(
                            s_ps[:, h2],
                            lhsT=kT[p][base:base + HD, c0:c0 + P],
                            rhs=qT[p][base:base + HD, q0:q0 + QB],
                            start=True, stop=True,
                            tile_position=(base, 0))
                    pt = ptp.tile([P, 2, QB], F32R, tag="pt")
                    nc.scalar.activation(out=pt[:], in_=s_ps[:],
                                         func=Exp if do_exp else
                                         mybir.ActivationFunctionType.Copy)
                    if pending_av is not None:
                        emit_av(*pending_av)
                    pending_av = (pt, c)
                emit_av(*pending_av)

                # normalize directly out of PSUM: reciprocal of each head's
                # sums row (aligned partitions 64 / 32), ones-broadcast via
                # K=1 matmuls into the matching partition range, then one
                # lane-aligned multiply per head.
                at_pair = atp.tile([P, QB], F32R, tag="attn")
                # evict everything needed from the av PSUM tiles right away
                # (sums rows + unnormalized dims) so the next pair's AV
                # accumulations reclaim the PSUM banks without waiting for
                # the slow reciprocal chain
                nc.vector.tensor_copy(out=sums_sb[HD:HD + 1, :],
                                      in_=av0[HD:HD + 1, :])
                nc.vector.tensor_copy(out=sums_sb[0:1, :],
                                      in_=av1[0:1, :])
                at_raw = bcp.tile([P, QB], F32, tag="araw")
                nc.vector.tensor_copy(out=at_raw[0:HD, :], in_=av0[0:HD, :])
                nc.vector.tensor_copy(out=at_raw[HD:P, :], in_=av1[HD:P, :])
                if do_norm:
                    # one batched reciprocal spanning rows 0..64 (DVE
                    # reciprocal cost is free-dim-bound: covering both sums
                    # rows plus filler costs the same as one row), then
                    # round to f32r (the matmul rhs needs an f32r producer)
                    nc.vector.reciprocal(out=rc_sb[0:HD + 1, :],
                                         in_=sums_sb[0:HD + 1, :])
                    nc.vector.tensor_copy(out=rcr_sb[HD:HD + 1, :],
                                          in_=rc_sb[HD:HD + 1, :])
                    nc.vector.tensor_copy(out=rcr_sb[0:1, :],
                                          in_=rc_sb[0:1, :])
                    rcr = rcr_sb
                    # broadcast each reciprocal row to all 128 partitions
                    # (matmul PSUM dst must start at partition 0), then copy
                    # the needed half to SBUF (DVE reads max one PSUM operand)
                    # deprioritized: the broadcasts depend on the (slow) DVE
                    # reciprocal chain; placed inline they would stall the
                    # next pair's score matmuls in the PE's in-order queue
                    bc_e = ps_y.tile([P, QB], F32, tag="y", name="bc_e")
                    bc_o = ps_y.tile([P, QB], F32, tag="y", name="bc_o")
                    with tc.high_priority(offset=-1000000):
                        nc.tensor.matmul(bc_e[:, :], lhsT=ones_even,
                                         rhs=rcr[HD:HD + 1, :],
                                         start=True, stop=True)
                        nc.tensor.matmul(bc_o[:, :], lhsT=ones_odd,
                                         rhs=rcr[0:1, :],
                                         start=True, stop=True)
                    nc.vector.tensor_tensor(out=at_pair[0:HD, :],
                                            in0=at_raw[0:HD, :],
                                            in1=bc_e[0:HD, :],
                                            op=mybir.AluOpType.mult)
                    nc.vector.tensor_tensor(out=at_pair[HD:P, :],
                                            in0=at_raw[HD:P, :],
                                            in1=bc_o[HD:P, :],
                                            op=mybir.AluOpType.mult)
                else:
                    nc.vector.tensor_copy(out=at_pair[0:HD, :],
                                          in_=at_raw[0:HD, :])
                    nc.vector.tensor_copy(out=at_pair[HD:P, :],
                                          in_=at_raw[HD:P, :])
                attn.append(at_pair)
                # remaining Q^T halves, one group per pass of block 1,
                # deprioritized so they only fill PE slack
                if do_proj and qb == 1:
                    with tc.high_priority(offset=-1000000):
                        emit_qk_group(wq_sb, qT, 0, p, 1)

            def emit_outproj_m(m, attn=attn, q0=q0):
                # one m-chunk of y^T[m, qb] = sum_p Wo_p^T @ attn_pair_p.
                # Deprioritized: these matmuls fill PE slack so they never
                # delay the score matmuls that feed the exp stream.
                with tc.high_priority(offset=-1000000):
                    yp = ps_y.tile([P, QB], F32, tag="y", name="yp")
                    for h in range(2):
                        nc.tensor.matmul(
                            yp[:],
                            lhsT=wo_sb[:, h, m * P:(m + 1) * P],
                            rhs=attn[h][:],
                            start=(h == 0), stop=(h == 1))
                    ysb = ysbp.tile([P, QB], F32, tag="ysb")
                    nc.vector.tensor_copy(out=ysb, in_=yp[:])
                    nc.sync.dma_start(out=yT[m * P:(m + 1) * P, q0:q0 + QB],
                                      in_=ysb)

            pending_outproj = [
                (lambda m=m: emit_outproj_m(m))
                for m in range(NDC if do_outproj else 0)]

        while pending_outproj:
            pending_outproj.pop(0)()

    nc.compile()
    return nc


_CACHE = {}


def _get_nc():
    if "nc" not in _CACHE:
        _CACHE["nc"] = _build_nc()
    return _CACHE["nc"]


def make_in_maps(x, Wq, bq, Wk, bk, Wv, bv, Wo, bo):
    """Host-side sharding: per-core input dicts for cores 0..7."""
    x = np.asarray(x, np.float32)
    scale = np.float32(1.0 / np.sqrt(HD))
    Wq_s = np.asarray(Wq, np.float32) * scale
    bq_s = np.asarray(bq, np.float32) * scale
    Wk = np.asarray(Wk, np.float32)
    bk = np.asarray(bk, np.float32)
    Wv = np.asarray(Wv, np.float32)
    Wo = np.asarray(Wo, np.float32)

    C = np.zeros((P, 768), np.float32)
    C[:, 0] = 1.0
    C[64, 640:640 + P] = 1.0
    C[0, 640:640 + P] = 1.0
    C[:, 740:744] = 1.0

    xts = [np.ascontiguousarray(x[b].T) for b in range(2)]
    in_maps = []
    for core in range(8):
        b, g = divmod(core, 4)
        cols = slice(g * GD, (g + 1) * GD)
        bias = np.zeros((P, 4), np.float32)
        bias[:, 0] = bq_s[g * GD:g * GD + P]
        bias[:, 1] = bq_s[g * GD + P:(g + 1) * GD]
        bias[:, 2] = bk[g * GD:g * GD + P]
        bias[:, 3] = bk[g * GD + P:(g + 1) * GD]
        in_maps.append({
            "cst": C,
            "xT": xts[b],
            "wq": np.ascontiguousarray(Wq_s[:, cols]),
            "wk": np.ascontiguousarray(Wk[:, cols]),
            "wv": np.ascontiguousarray(Wv[:, cols]),
            "wo": np.ascontiguousarray(Wo[cols, :]),
            "bias": bias,
        })
    return in_maps


def gather_output(results, Wv, bv, Wo, bo):
    """Sum per-core partial y^T outputs and fold bv/bo exactly."""
    y = np.zeros((2, S, D), np.float32)
    for core in range(8):
        b = core // 4
        y[b] += results[core]["yT"].T
    y += np.asarray(bo, np.float32) + np.asarray(bv, np.float32) @ np.asarray(Wo, np.float32)
    return y


def kernel(x, Wq, bq, Wk, bk, Wv, bv, Wo, bo):
    global LAST_RESULTS
    from concourse.bass_utils import run_bass_kernel_spmd
    in_maps = make_in_maps(x, Wq, bq, Wk, bk, Wv, bv, Wo, bo)
    res = run_bass_kernel_spmd(_get_nc(), in_maps, core_ids=list(range(8)),
                               trace=TRACE)
    LAST_RESULTS = res
    return gather_output(res.results, Wv, bv, Wo, bo)


# revision 29
# speedup vs baseline: 1.1208x; 1.1208x over previous
"""Trainium2 Bass kernel for a 16-head self-attention block.

Model (matches the nn.Module reference):
    q = x @ Wq + bq; k = x @ Wk + bk; v = x @ Wv + bv   (per-head split, Hd=64)
    attn = softmax(q k^T / sqrt(Hd)); out = (attn v) @ Wo + bo
Shapes: x [2, 2048, 1024], 16 heads, head dim 64.

Sharding (8 cores): core = (batch b in {0,1}) x (head-group g in {0..3});
each core owns 4 heads of one batch element. Inputs are sliced on the host;
each core returns a partial y^T = (attended_g @ Wo_g)^T which the host sums
over the 4 head-groups per batch.

Per-core design:
  - Host passes xT = x[b]^T in bf16 (as are all weights), halving the
    DMA-bound prologue; weights are pre-rearranged on the host so every DMA
    moves contiguous partition lines. Projections run in bf16 (psum stays
    f32); K^T/Q^T are evicted as f32r so the big score matmuls run full
    precision on the fast PE path.
  - Scores are computed transposed, S^T[key, q] = K_h Q_h^T, so softmax's
    exp runs straight out of PSUM on the Scalar engine and A = P V consumes
    P^T with no transpose anywhere. Two heads of a pair share each score
    slab via PE row groups (K=64 at row offsets 0/64).
  - softmax skips the max subtraction (mathematically identical; scores are
    O(5) here and ACT exp is <=2 ULP on [-10,10]).
  - The Scalar engine runs ONLY the exp stream (the critical path, ~1.1us
    per [128,1024] eviction); K/Q projection evictions (+bias) run on DVE
    via tensor_scalar. exp writes P^T in bf16, feeding bf16 AV matmuls.
  - P row sums come from ones columns embedded in the V slab: the even head
    uses lhsT [d(64)|1] (M=65, sums at PSUM row 64); the odd head uses the
    128-wide window starting at the even ones column (M=128, so the ones
    land at row 0: sums at row 0, dims at rows 64-127, filler columns in
    between produce unread rows). Both sums rows land on 0/64, so one
    batched DVE reciprocal (free-dim-bound: rows 0..64 cost the same as one
    row) + two K=1 ones-broadcast matmuls + two lane-aligned multiplies
    normalize everything -- no partition-shift DMAs.
  - The attention loop is software-pipelined by one chunk (scores(c)+exp(c)
    emitted before AV(c-1)) and the attended dims are copied out of PSUM
    immediately at pair end so the next pair's AV accumulation never waits
    on the reciprocal chain. Output-projection m-chunks are spread one per
    key chunk to fill PE slack without stalling the exp stream. Projection
    PSUM comes from the ps_y pool so the score-slab pool's ring order never
    serializes the first query block behind the remaining projections.
  - 1/sqrt(Hd) is folded into Wq (and bq) on the host; bv and bo are folded
    in exactly on the host: y += bo + bv @ Wo (softmax rows sum to 1).
"""

import numpy as np
import ml_dtypes

import concourse.bass as bass
import concourse.tile as tile
from concourse import bacc
from concourse import mybir

P = 128          # partitions
S = 2048         # sequence length
D = 1024         # model dim
H = 16           # total heads
HD = 64          # head dim
G = 4            # heads per core
GD = G * HD      # 256 head-group dims per core
NQB = 4          # query blocks
QB = S // NQB    # 512
NKC = S // P     # 16 key chunks
NDC = D // P     # 8 contraction chunks
VW = 192         # per-pair V slab width: [d0(64),1,filler(63),d1(64)]
F32 = mybir.dt.float32
F32R = mybir.dt.float32r
BF16 = mybir.dt.bfloat16

TRACE = False
LAST_RESULTS = None


def _build_nc(nqb=NQB, do_attn=True, do_exp=True, do_outproj=True,
              do_norm=True, do_proj=True, do_dma=True):
    nc = bacc.Bacc(trn_type="TRN2")
    xT = nc.dram_tensor("xT", [D, S], BF16, kind="ExternalInput")
    wq = nc.dram_tensor("wq", [P, NDC, GD], BF16, kind="ExternalInput")
    wk = nc.dram_tensor("wk", [P, NDC, GD], BF16, kind="ExternalInput")
    wv = nc.dram_tensor("wv", [P, NDC, GD], BF16, kind="ExternalInput")
    wo = nc.dram_tensor("wo", [P, 2, D], BF16, kind="ExternalInput")
    bias = nc.dram_tensor("bias", [P, 4], F32, kind="ExternalInput")
    cst = nc.dram_tensor("cst", [P, 768], F32R, kind="ExternalInput")
    yT = nc.dram_tensor("yT", [D, S], F32, kind="ExternalOutput")

    Exp = mybir.ActivationFunctionType.Exp

    with tile.TileContext(nc) as tc, \
         tc.tile_pool(name="sb", bufs=1) as sb, \
         tc.tile_pool(name="pt", bufs=3) as ptp, \
         tc.tile_pool(name="attnp", bufs=5) as atp, \
         tc.tile_pool(name="ysbp", bufs=2) as ysbp, \
         tc.tile_pool(name="bcp", bufs=2) as bcp, \
         tc.tile_pool(name="ps_s", bufs=2, space="PSUM") as ps_s, \
         tc.tile_pool(name="ps_av", bufs=2, space="PSUM") as ps_av, \
         tc.tile_pool(name="ps_y", bufs=2, space="PSUM") as ps_y:

        # ---- persistent SBUF tensors
        wq_sb = sb.tile([P, NDC, GD], BF16, tag="wq")
        wk_sb = sb.tile([P, NDC, GD], BF16, tag="wk")
        wv_sb = sb.tile([P, NDC, GD], BF16, tag="wv")
        wo_sb = sb.tile([P, 2, D], BF16, tag="wo")   # [pair-dims, pair, out-dim]
        bias_sb = sb.tile([P, 4], F32, tag="bias")
        scratch = sb.tile([P, 1], F32, tag="scratch")
        cst_sb = sb.tile([P, 768], F32R, tag="cst")
        ones_col = cst_sb[:, 0:1]                    # [128, 1] ones
        ones_even = cst_sb[64:65, 640:640 + P]       # [1, 128] ones at row 64
        ones_odd = cst_sb[0:1, 640:640 + P]          # [1, 128] ones at row 0
        x_sb = [sb.tile([P, S], BF16, tag=f"x{d}", name=f"x{d}") for d in range(NDC)]
        kT = [sb.tile([P, S], F32R, tag=f"k{p}", name=f"k{p}") for p in range(2)]
        qT = [sb.tile([P, S], F32R, tag=f"q{p}", name=f"q{p}") for p in range(2)]
        # V slab per key chunk: [keys, pair, VW]:
        # cols 0:64 even dims, 64 ones, 65:128 filler(1.0), 128:192 odd dims
        v_sb = [sb.tile([P, 2, VW], BF16, tag=f"v{c}", name=f"v{c}")
                for c in range(NKC)]
        # persistent normalization staging; filler rows stay memset
        sums_sb = sb.tile([P, QB], F32, tag="sums")
        rc_sb = sb.tile([P, QB], F32, tag="rcp")
        rcr_sb = sb.tile([P, QB], F32R, tag="rcr")

        # ---- input DMAs
        if do_dma:
            nc.sync.dma_start(out=wk_sb, in_=wk[:, :, :])
            nc.sync.dma_start(out=bias_sb, in_=bias[:, :])
            nc.sync.dma_start(out=wq_sb, in_=wq[:, :, :])
            for d in range(NDC):
                nc.sync.dma_start(out=x_sb[d], in_=xT[d * P:(d + 1) * P, :])
            nc.sync.dma_start(out=wv_sb, in_=wv[:, :, :])
            nc.sync.dma_start(out=cst_sb, in_=cst[:, :])
            nc.sync.dma_start(out=wo_sb, in_=wo[:, :, :])
        # warm the exp table set early so the ~2.7us load overlaps the prologue
        nc.scalar.activation(out=scratch, in_=ones_col.bitcast(F32), func=Exp)

        # pre-fill V slabs with 1.0 (DVE is idle during the DMA-bound
        # prologue): provides the per-head ones columns and initializes the
        # filler columns of each 128-wide odd-head window
        for c in range(NKC):
            nc.vector.memset(v_sb[c][:], 1.0)
        nc.vector.memset(sums_sb[:], 1.0)

        # Pre-observe each weight DMA on the PE with a 1x1 dummy matmul, so
        # real matmuls never need two DMA-queue waits at once (walrus can't
        # encode >1 sync wait on an LDWEIGHTS).
        wtouch_ps = ps_y.tile([1, 4], F32, tag="y", name="wtouch")
        for i, w in enumerate((wk_sb, wv_sb, wq_sb, wo_sb)):
            nc.tensor.matmul(wtouch_ps[:, i:i + 1],
                             lhsT=w[0:1, 0, 0:1],
                             rhs=w[0:1, 0, 0:1],
                             start=True, stop=True)

        # ---- projection emitters (PSUM from the ps_y pool: the ps_s pool
        # must serve ONLY score slabs, or its ring order would serialize the
        # first query block behind every remaining projection group)
        def emit_qk_half(w_sb, dst, bcol0, p, nb):
            # one [128, 512] slab of K^T or Q^T; dst[p] [128, 2048] rows
            # 64*h2 hold head (2p+h2)'s dims, columns are sequence n.
            ps = ps_y.tile([P, QB], F32, tag="y", name="qk_ps")
            n0 = nb * QB
            for d in range(NDC):
                nc.tensor.matmul(
                    ps[:],
                    lhsT=w_sb[:, d, p * P:(p + 1) * P],
                    rhs=x_sb[d][:, n0:n0 + QB],
                    start=(d == 0), stop=(d == NDC - 1))
            # evict with per-partition bias add on DVE (keeps ACT free)
            nc.vector.tensor_scalar_add(
                out=dst[p][:, n0:n0 + QB],
                in0=ps[:],
                scalar1=bias_sb[:, bcol0 + p:bcol0 + p + 1])

        def emit_v_chunk(c):
            ps = ps_y.tile([P, GD], F32, tag="y", name="v_ps")
            for d in range(NDC):
                nc.tensor.matmul(
                    ps[:],
                    lhsT=x_sb[d][:, c * P:(c + 1) * P],
                    rhs=wv_sb[:, d, :],
                    start=(d == 0), stop=(d == NDC - 1))
            pshd = ps[:].rearrange("p (a b d) -> p a b d", a=2, b=2)
            # even heads (h2=0) -> cols 0:64; odd heads (h2=1) -> cols
            # 128:192 (slab pre-filled with 1.0 for the ones/filler columns)
            nc.vector.tensor_copy(out=v_sb[c][:, :, 0:HD], in_=pshd[:, :, 0, :])
            nc.vector.tensor_copy(out=v_sb[c][:, :, 2 * HD:3 * HD],
                                  in_=pshd[:, :, 1, :])

        if do_proj:
            # K first (scores chunk c needs kT columns c*128..), then the Q
            # blocks for qb0, the rest of K, Q for qb1, then all V chunks
            # (they fill the PE slack of the DMA-bound prologue tail).
            for p in range(2):
                for nb in range(2):
                    emit_qk_half(wk_sb, kT, 2, p, nb)
            for p in range(2):
                emit_qk_half(wq_sb, qT, 0, p, 0)
            for p in range(2):
                for nb in range(2, 4):
                    emit_qk_half(wk_sb, kT, 2, p, nb)
            for p in range(2):
                emit_qk_half(wq_sb, qT, 0, p, 1)
            for c in range(NKC):
                emit_v_chunk(c)
        # Q slabs for qb2/qb3 are deferred into the qb1 passes (emitted
        # deprioritized there so they only fill PE slack)

        # ---- attention + output projection
        pending_outproj = []
        for qb in range(nqb if do_attn else 0):
            q0 = qb * QB
            attn = []
            for p in range(2):
                # av0: even head, M=65, dims rows 0-63, sums row 64.
                # av1: odd head, M=128, sums row 0 (window col 0 is the even
                # head's ones column), dims rows 64-127.
                av0 = ps_av.tile([P, QB], F32, tag="av", name="av0")
                av1 = ps_av.tile([P, QB], F32, tag="av", name="av1")

                def emit_av(pt, c, av0=av0, av1=av1, p=p):
                    nc.tensor.matmul(
                        av0[0:HD + 1, :],
                        lhsT=v_sb[c][:, p, 0:HD + 1],
                        rhs=pt[:, 0],
                        start=(c == 0), stop=(c == NKC - 1))
                    nc.tensor.matmul(
                        av1[:, :],
                        lhsT=v_sb[c][:, p, HD:HD + P],
                        rhs=pt[:, 1],
                        start=(c == 0), stop=(c == NKC - 1))

                # software-pipelined by one chunk: scores(c)+exp(c) are
                # emitted BEFORE av(c-1) so the (independent) score matmuls
                # of chunk c can run while the Scalar engine computes
                # exp(c-1); the exp stream then runs back-to-back.
                pending_av = None
                for c in range(NKC):
                    if pending_outproj and p == 0 and c >= 2:
                        # one output-projection m-chunk per key chunk: its 2
                        # matmuls fit the per-chunk PE slack, where a clump
                        # would stall the exp stream for ~8us
                        pending_outproj.pop(0)()
                    c0 = c * P
                    s_ps = ps_s.tile([P, 2, QB], F32, tag="s")
                    for h2 in range(2):
                        base = HD * h2
                        nc.tensor.matmul(
                            s_ps[:, h2],
                            lhsT=kT[p][base:base + HD, c0:c0 + P],
                            rhs=qT[p][base:base + HD, q0:q0 + QB],
                            start=True, stop=True,
                            tile_position=(base, 0))
                    pt = ptp.tile([P, 2, QB], BF16, tag="pt")
                    nc.scalar.activation(out=pt[:], in_=s_ps[:],
                                         func=Exp if do_exp else
                                         mybir.ActivationFunctionType.Copy)
                    if pending_av is not None:
                        emit_av(*pending_av)
                    pending_av = (pt, c)
                emit_av(*pending_av)

                # evict everything needed from the av PSUM tiles right away
                # (sums rows + unnormalized dims) so the next pair's AV
                # accumulations reclaim the PSUM banks without waiting for
                # the slow reciprocal chain
                at_pair = atp.tile([P, QB], BF16, tag="attn")
                nc.vector.tensor_copy(out=sums_sb[HD:HD + 1, :],
                                      in_=av0[HD:HD + 1, :])
                nc.vector.tensor_copy(out=sums_sb[0:1, :],
                                      in_=av1[0:1, :])
                at_raw = bcp.tile([P, QB], F32, tag="araw")
                nc.vector.tensor_copy(out=at_raw[0:HD, :], in_=av0[0:HD, :])
                nc.vector.tensor_copy(out=at_raw[HD:P, :], in_=av1[HD:P, :])
                if do_norm:
                    # one batched reciprocal spanning rows 0..64 (DVE
                    # reciprocal cost is free-dim-bound: both sums rows plus
                    # filler cost the same as one row), then round to f32r
                    # (the broadcast matmul rhs needs an f32r producer)
                    nc.vector.reciprocal(out=rc_sb[0:HD + 1, :],
                                         in_=sums_sb[0:HD + 1, :])
                    nc.vector.tensor_copy(out=rcr_sb[HD:HD + 1, :],
                                          in_=rc_sb[HD:HD + 1, :])
                    nc.vector.tensor_copy(out=rcr_sb[0:1, :],
                                          in_=rc_sb[0:1, :])
                    # broadcast each reciprocal row to all 128 partitions
                    # (matmul PSUM dst must start at partition 0);
                    # deprioritized: they depend on the slow reciprocal
                    # chain and must not stall the next pair's scores
                    bc_e = ps_y.tile([P, QB], F32, tag="y", name="bc_e")
                    bc_o = ps_y.tile([P, QB], F32, tag="y", name="bc_o")
                    with tc.high_priority(offset=-1000000):
                        nc.tensor.matmul(bc_e[:, :], lhsT=ones_even,
                                         rhs=rcr_sb[HD:HD + 1, :],
                                         start=True, stop=True)
                        nc.tensor.matmul(bc_o[:, :], lhsT=ones_odd,
                                         rhs=rcr_sb[0:1, :],
                                         start=True, stop=True)
                    nc.vector.tensor_tensor(out=at_pair[0:HD, :],
                                            in0=at_raw[0:HD, :],
                                            in1=bc_e[0:HD, :],
                                            op=mybir.AluOpType.mult)
                    nc.vector.tensor_tensor(out=at_pair[HD:P, :],
                                            in0=at_raw[HD:P, :],
                                            in1=bc_o[HD:P, :],
                                            op=mybir.AluOpType.mult)
                else:
                    nc.vector.tensor_copy(out=at_pair[0:HD, :],
                                          in_=at_raw[0:HD, :])
                    nc.vector.tensor_copy(out=at_pair[HD:P, :],
                                          in_=at_raw[HD:P, :])
                attn.append(at_pair)
                # remaining Q^T slabs, one per pass of block 1,
                # deprioritized so they only fill PE slack
                if do_proj and qb == 1:
                    with tc.high_priority(offset=-1000000):
                        emit_qk_half(wq_sb, qT, 0, p, 2)
                        emit_qk_half(wq_sb, qT, 0, p, 3)

            def emit_outproj_m(m, attn=attn, q0=q0):
                # one m-chunk of y^T[m, qb] = sum_p Wo_p^T @ attn_pair_p.
                # Deprioritized: these matmuls fill PE slack so they never
                # delay the score matmuls that feed the exp stream.
                with tc.high_priority(offset=-1000000):
                    yp = ps_y.tile([P, QB], F32, tag="y", name="yp")
                    for h in range(2):
                        nc.tensor.matmul(
                            yp[:],
                            lhsT=wo_sb[:, h, m * P:(m + 1) * P],
                            rhs=attn[h][:],
                            start=(h == 0), stop=(h == 1))
                    ysb = ysbp.tile([P, QB], F32, tag="ysb")
                    nc.vector.tensor_copy(out=ysb, in_=yp[:])
                    nc.sync.dma_start(out=yT[m * P:(m + 1) * P, q0:q0 + QB],
                                      in_=ysb)

            pending_outproj = [
                (lambda m=m: emit_outproj_m(m))
                for m in range(NDC if do_outproj else 0)]

        while pending_outproj:
            pending_outproj.pop(0)()

    nc.compile()
    return nc


_CACHE = {}


def _get_nc():
    if "nc" not in _CACHE:
        _CACHE["nc"] = _build_nc()
    return _CACHE["nc"]


def make_in_maps(x, Wq, bq, Wk, bk, Wv, bv, Wo, bo):
    """Host-side sharding: per-core input dicts for cores 0..7."""
    bf16 = ml_dtypes.bfloat16
    x = np.asarray(x, np.float32)
    scale = np.float32(1.0 / np.sqrt(HD))
    Wq_s = np.asarray(Wq, np.float32) * scale
    bq_s = np.asarray(bq, np.float32) * scale
    Wk = np.asarray(Wk, np.float32)
    bk = np.asarray(bk, np.float32)
    Wv = np.asarray(Wv, np.float32)
    Wo = np.asarray(Wo, np.float32)

    C = np.zeros((P, 768), np.float32)
    C[:, 0] = 1.0
    C[64, 640:640 + P] = 1.0
    C[0, 640:640 + P] = 1.0

    def warr(w):   # [D, cols] -> [P, NDC, cols] partition-major, contiguous
        return np.ascontiguousarray(
            w.reshape(NDC, P, -1).transpose(1, 0, 2).astype(bf16))

    xts = [np.ascontiguousarray(x[b].T.astype(bf16)) for b in range(2)]
    in_maps = []
    for core in range(8):
        b, g = divmod(core, 4)
        cols = slice(g * GD, (g + 1) * GD)
        bias = np.zeros((P, 4), np.float32)
        bias[:, 0] = bq_s[g * GD:g * GD + P]
        bias[:, 1] = bq_s[g * GD + P:(g + 1) * GD]
        bias[:, 2] = bk[g * GD:g * GD + P]
        bias[:, 3] = bk[g * GD + P:(g + 1) * GD]
        # wo: [GD, D] -> [P, 2, D]: pair h rows (h*128..h*128+127)
        wo_c = np.ascontiguousarray(
            Wo[cols, :].reshape(2, P, D).transpose(1, 0, 2).astype(bf16))
        in_maps.append({
            "cst": C,
            "xT": xts[b],
            "wq": warr(Wq_s[:, cols]),
            "wk": warr(Wk[:, cols]),
            "wv": warr(Wv[:, cols]),
            "wo": wo_c,
            "bias": bias,
        })
    return in_maps


def gather_output(results, Wv, bv, Wo, bo):
    """Sum per-core partial y^T outputs and fold bv/bo exactly."""
    y = np.zeros((2, S, D), np.float32)
    for core in range(8):
        b = core // 4
        y[b] += results[core]["yT"].T
    y += np.asarray(bo, np.float32) + np.asarray(bv, np.float32) @ np.asarray(Wo, np.float32)
    return y


def kernel(x, Wq, bq, Wk, bk, Wv, bv, Wo, bo):
    global LAST_RESULTS
    from concourse.bass_utils import run_bass_kernel_spmd
    in_maps = make_in_maps(x, Wq, bq, Wk, bk, Wv, bv, Wo, bo)
    res = run_bass_kernel_spmd(_get_nc(), in_maps, core_ids=list(range(8)),
                               trace=TRACE)
    LAST_RESULTS = res
    return gather_output(res.results, Wv, bv, Wo, bo)
